# revision 1
# baseline (speedup 1.0000x reference)
"""Mamba block kernel for 8 Trainium2 NeuronCores (Bass/Tile, SPMD).

Sharding: 2-way data-parallel over batch x 4-way tensor-parallel over
d_inner. Core c handles batch c//4 and channel block c%4 (512 channels).

Per-core structure, software-pipelined over two L/2 halves so h1's
projection/conv work overlaps h0's scan (engines stream per-FIFO, so the
overlap is realized by interleaving emission via deferred "slot"
closures):

  prologue: PE-transpose hs (bf16) -> in_proj h0+h1-x (bf16 matmuls,
  SiLU(z) fused into the PSUM evacuation) -> depthwise conv (h0 on the
  idle DVE via stt chains, h1 on PE via diag matmuls) -> x_dbl partial
  + per-half 4-core AllReduce (bf16) -> delta = softplus(dt proj).

  scan half h: per state-dim n: one stride-0 broadcast DMA fetches the
  interleaved B_n/C_n rows; dA = exp(delta*A_n) on ScalarE (fp32 - its
  error compounds over the decay length; everything else rides bf16);
  dBu = du*B_n on GpSimd (two of 16 state dims on DVE for balance);
  native DVE tensor_tensor_scan recurrence (halves chained through a
  carry strip written by SBUF->SBUF DMAs on the SP queue, off the Act
  stream); sC = s*C_n on DVE (bf16 2x mode); y = sum_n C_n*s_n + D*u
  accumulated in PSUM via identity/diag matmuls on TensorE; gate
  yg = y * SiLU(z), deferred past the blk-pair boundary.

  out_proj (bf16) streams into E(h1)'s PE gaps for h0, then the tail
  finishes h1 -> 4-core ReduceScatter (bf16).

The scalar engine's activation tables are pinned: all SiLU work happens
in the prologue, a manual LoadActFuncSet (with an explicit dependency so
the scheduler cannot hoist it) then selects natural_log_exp_and_others,
which covers the whole softplus/exp/copy mix of the scan windows.
"""
import numpy as np

BATCH, L, D_MODEL = 2, 2048, 1024
D_INNER, D_STATE, D_CONV, DT_RANK = 2048, 16, 4, 64
NCORES = 8
DC = D_INNER // 4          # 512 channels per core
NBLK = DC // 128           # 4 partition blocks
P = 128
NQ = L // 512              # 4 time quarters for matmul N-tiling
H = L // 2                 # scan half length

TRACE = False              # set by test.py to capture a profile
LAST_RESULTS = None        # BassKernelResults stash for test.py

_prog_cache = {}


class _Stop(Exception):
    pass


def _build_program(collectives=True, upto="F"):
    lvl = {"A": 1, "B": 2, "D": 3, "E": 4, "F": 5}[upto]
    import concourse.bass as bass
    import concourse.mybir as mybir
    import concourse.tile as tile
    from concourse import bacc
    from concourse.masks import make_identity
    from concourse.hw_specs import get_activation_tables
    from contextlib import ExitStack

    f32 = mybir.dt.float32
    f32r = mybir.dt.float32r
    bf16 = mybir.dt.bfloat16
    MULT = mybir.AluOpType.mult
    ADD = mybir.AluOpType.add
    ACT = mybir.ActivationFunctionType

    nc = bacc.Bacc("TRN2", target_bir_lowering=False, debug=False,
                   num_devices=NCORES)

    # ---- kernel I/O (per-core shapes; host pre-arranges layouts) ----
    hsb = nc.dram_tensor("hsb", [L, D_MODEL], f32, kind="ExternalInput")
    wxz = nc.dram_tensor("wxz", [P, 8, 1024], f32, kind="ExternalInput")
    convw = nc.dram_tensor("convw", [P, NBLK * D_CONV], f32, kind="ExternalInput")
    convb = nc.dram_tensor("convb", [P, NBLK], f32, kind="ExternalInput")
    xprojT = nc.dram_tensor("xprojT", [P, NBLK, 96], f32, kind="ExternalInput")
    dtprojT = nc.dram_tensor("dtprojT", [DT_RANK, DC], f32, kind="ExternalInput")
    dtb = nc.dram_tensor("dtb", [P, NBLK], f32, kind="ExternalInput")
    alog = nc.dram_tensor("alog", [P, NBLK * D_STATE], f32, kind="ExternalInput")
    dvec = nc.dram_tensor("dvec", [P, NBLK], f32, kind="ExternalInput")
    outw = nc.dram_tensor("outw", [P, NBLK, 1024], f32, kind="ExternalInput")
    out_shard = nc.dram_tensor("out_shard", [D_MODEL // 4, L], bf16,
                               kind="ExternalOutput")

    # ---- internal DRAM ----
    xdbl_p = [nc.dram_tensor(f"xdbl_p{h}", [96, H], bf16) for h in range(2)]
    xdbl_s = [nc.dram_tensor(f"xdbl_s{h}", [96, H], bf16) for h in range(2)]
    outpart = nc.dram_tensor("outpart", [D_MODEL, L], bf16)
    outrs = nc.dram_tensor("outrs", [D_MODEL // 4, L], bf16)

    GROUPS = [[0, 1, 2, 3], [4, 5, 6, 7]]

    with tile.TileContext(nc) as tc, ExitStack() as top:
        const_pool = top.enter_context(tc.tile_pool(name="const", bufs=1))
        big_pool = top.enter_context(tc.tile_pool(name="bigpool", bufs=1))
        ps_all = top.enter_context(
            tc.tile_pool(name="ps_all", bufs=4, space="PSUM"))
        ps_y = top.enter_context(
            tc.tile_pool(name="ps_y", bufs=2, space="PSUM"))

        # ---------------- constants ----------------
        ident = const_pool.tile([P, P], f32)
        make_identity(nc, ident)
        ident16 = const_pool.tile([P, P], bf16)
        nc.scalar.copy(ident16[:], ident[:])

        convw_sb = const_pool.tile([P, NBLK * D_CONV], f32)
        nc.sync.dma_start(convw_sb[:], convw[:])
        convb_sb = const_pool.tile([P, NBLK], f32)
        nc.sync.dma_start(convb_sb[:], convb[:])
        dtb_sb = const_pool.tile([P, NBLK], f32)
        nc.sync.dma_start(dtb_sb[:], dtb[:])
        dvec_sb = const_pool.tile([P, NBLK], f32)
        nc.sync.dma_start(dvec_sb[:], dvec[:])
        alog_sb = const_pool.tile([P, NBLK * D_STATE], f32)
        nc.sync.dma_start(alog_sb[:], alog[:])
        negA = const_pool.tile([P, NBLK * D_STATE], f32)
        nc.scalar.activation(negA[:], alog_sb[:], ACT.Exp)
        nc.scalar.mul(negA[:], negA[:], -1.0)
        # diag(D) per block + diag(conv_w) per (block, tap), bf16
        diagD = const_pool.tile([P, NBLK, P], bf16)
        diagCW = const_pool.tile([P, NBLK * D_CONV, P], bf16)
        for blk in range(NBLK):
            dtmp = const_pool.tile([P, P], f32, tag="dtmp", name="dtmp")
            nc.vector.tensor_scalar_mul(dtmp[:], ident[:],
                                        dvec_sb[:, blk:blk + 1])
            nc.scalar.copy(diagD[:, blk, :], dtmp[:])
            for w in range(D_CONV):
                ctmp = const_pool.tile([P, P], f32, tag="ctmp", name="ctmp")
                nc.vector.tensor_scalar_mul(
                    ctmp[:], ident[:],
                    convw_sb[:, blk * D_CONV + w: blk * D_CONV + w + 1])
                nc.scalar.copy(diagCW[:, blk * D_CONV + w, :], ctmp[:])

        # ---------------- persistent activations ----------------
        zsil = big_pool.tile([P, NBLK, L], bf16)    # SiLU(z)
        yg = big_pool.tile([P, NBLK, L], bf16)      # gated scan output
        hsT16 = big_pool.tile([P, 8, L], bf16)      # transposed hs
        w16 = big_pool.tile([P, 8, 1024], bf16)     # in_proj weights
        # conv input x, one half at a time: [3-tap halo + pad | H cols]
        xp = [big_pool.tile([P, 4 + H], bf16, name=f"xp{b}")
              for b in range(NBLK)]
        u16 = big_pool.tile([P, NBLK, L], bf16)     # conv+SiLU output
        delta = big_pool.tile([P, NBLK, L], bf16)
        du = big_pool.tile([P, NBLK, L], bf16)
        carry = big_pool.tile([P, NBLK * D_STATE], bf16)
        xw_b = big_pool.tile([P, NBLK, 96], bf16)
        dtw_b = big_pool.tile([DT_RANK, DC], bf16)
        ow_b = big_pool.tile([P, NBLK, 1024], bf16)
        dtr = [big_pool.tile([DT_RANK, H], bf16, name=f"dtr{h}")
               for h in range(2)]

        for blk in range(NBLK):
            nc.vector.memset(xp[blk][:, 0:4], 0.0)

        # ---------------- weight staging ----------------
        with tc.tile_pool(name="wstage", bufs=2) as w_stage:
            for kb in range(8):
                w_sb = w_stage.tile([P, 1024], f32, tag="w", name=f"w{kb}")
                nc.gpsimd.dma_start(w_sb[:], wxz[:, kb, :])
                nc.scalar.copy(w16[:, kb, :], w_sb[:])
            xw_sb = w_stage.tile([P, NBLK, 96], f32, tag="xw", name="xw")
            nc.gpsimd.dma_start(xw_sb[:], xprojT[:])
            nc.scalar.copy(xw_b[:], xw_sb[:])
            dtw_sb = w_stage.tile([DT_RANK, DC], f32, tag="dtw", name="dtw")
            nc.gpsimd.dma_start(dtw_sb[:], dtprojT[:])
            nc.scalar.copy(dtw_b[:], dtw_sb[:])
            for kb in range(NBLK):
                ow_sb = w_stage.tile([P, 1024], f32, tag="ow",
                                     name=f"ow{kb}")
                nc.gpsimd.dma_start(ow_sb[:], outw[:, kb, :])
                nc.scalar.copy(ow_b[:, kb, :], ow_sb[:])

        # ---------------- transient pools ----------------
        xe_pool = top.enter_context(tc.tile_pool(name="xepool", bufs=2))
        sp_pool = top.enter_context(tc.tile_pool(name="sppool", bufs=2))
        # bc/da/db/s pools are opened mid-schedule, once the prologue's
        # scoped staging pools have released their SBUF.
        bc_pool = da_pool = db_pool = s_pool = None
        cv_box = [None]
        ot_pool = top.enter_context(tc.tile_pool(name="otpool", bufs=2))

        # ---------------- phase emitters ----------------
        def inproj_group(tq, mb, raw_z=False):
            half, tl = tq // 2, (tq % 2) * 512
            def emit():
                ps = ps_all.tile([P, 512], f32, tag="ps", name="psa")
                for kb in range(8):
                    nc.tensor.matmul(
                        ps[:], w16[:, kb, mb * P:(mb + 1) * P],
                        hsT16[:, kb, tq * 512:(tq + 1) * 512],
                        start=(kb == 0), stop=(kb == 7))
                if mb < 4:
                    nc.scalar.copy(xp[mb][:, 4 + tl: 4 + tl + 512], ps[:])
                elif raw_z:
                    # plain copy keeps the Act table set unchanged; the
                    # SiLU is applied in-place as a batch at E(h1) entry.
                    nc.scalar.copy(
                        zsil[:, mb - 4, tq * 512:(tq + 1) * 512], ps[:])
                else:
                    last_silu[0] = nc.scalar.activation(
                        zsil[:, mb - 4, tq * 512:(tq + 1) * 512],
                        ps[:], ACT.Silu)
            return emit

        def halo_copy(blk):
            # carry the last 3 x columns of h0 into h1's halo slot
            def emit():
                nc.scalar.copy(xp[blk][:, 1:4], xp[blk][:, 1 + H:4 + H])
            return emit

        last_silu = [None]

        def conv_group(tq, blk):
            half, tl = tq // 2, (tq % 2) * 512
            def emit():
                pc = ps_all.tile([P, 512], f32, tag="ps", name="psc")
                for w in range(D_CONV):
                    nc.tensor.matmul(
                        pc[:], diagCW[:, blk * D_CONV + w, :],
                        xp[blk][:, 1 + w + tl: 513 + w + tl],
                        start=(w == 0), stop=(w == D_CONV - 1))
                last_silu[0] = nc.scalar.activation(
                    u16[:, blk, tq * 512:(tq + 1) * 512], pc[:],
                    ACT.Silu, bias=convb_sb[:, blk:blk + 1])
            return emit

        def conv_dve(half, blk):
            # stt-chain conv for h0: runs on the DVE, which is idle during
            # the prologue, keeping the PE free for in_proj.
            h0 = half * H
            def emit():
                acc = cv_box[0].tile([P, H], bf16, tag="cv", name="cv")
                nc.vector.tensor_scalar_mul(
                    acc[:], xp[blk][:, 4:4 + H],
                    convw_sb[:, blk * 4 + 3: blk * 4 + 4])
                for w in (2, 1, 0):
                    nc.vector.scalar_tensor_tensor(
                        acc[:], xp[blk][:, 1 + w: 1 + w + H],
                        convw_sb[:, blk * 4 + w: blk * 4 + w + 1],
                        acc[:], MULT, ADD)
                last_silu[0] = nc.scalar.activation(
                    u16[:, blk, h0:h0 + H], acc[:],
                    ACT.Silu, bias=convb_sb[:, blk:blk + 1])
            return emit

        def xdbl_group(tq):
            half, tl = tq // 2, (tq % 2) * 512
            def emit():
                ps = ps_all.tile([P, 512], f32, tag="ps", name="psx")
                for kb in range(NBLK):
                    nc.tensor.matmul(ps[0:96, :], xw_b[:, kb, :],
                                     u16[:, kb, tq * 512:(tq + 1) * 512],
                                     start=(kb == 0), stop=(kb == 3))
                xe = xe_pool.tile([P, 512], bf16, tag="xe", name="xe")
                nc.scalar.copy(xe[0:96, :], ps[0:96, :])
                nc.sync.dma_start(xdbl_p[half][:, tl:tl + 512], xe[0:96, :])
            return emit

        def emit_ar(half):
            if collectives:
                nc.gpsimd.collective_compute(
                    "AllReduce", ADD, replica_groups=GROUPS,
                    ins=[xdbl_p[half][:]], outs=[xdbl_s[half][:]])
            else:
                nc.sync.dma_start(xdbl_s[half][:], xdbl_p[half][:])
            nc.sync.dma_start(dtr[half][:], xdbl_s[half][0:DT_RANK, :])

        def dt_group(tq, blk):
            half, tl = tq // 2, (tq % 2) * 512
            def emit():
                ps = ps_all.tile([P, 512], f32, tag="ps", name="psd")
                nc.tensor.matmul(
                    ps[:], dtw_b[:, blk * P:(blk + 1) * P],
                    dtr[half][:, tl:tl + 512])
                # softplus(x+b) = relu(x+b) + ln(1+exp(-|x+b|))
                ta = sp_pool.tile([P, 512], bf16, tag="spA", name="ta")
                nc.scalar.activation(ta[:], ps[:], ACT.Abs,
                                     bias=dtb_sb[:, blk:blk + 1])
                te = sp_pool.tile([P, 512], bf16, tag="spB", name="te")
                nc.scalar.activation(te[:], ta[:], ACT.Exp, scale=-1.0)
                tlg = sp_pool.tile([P, 512], bf16, tag="spA", name="tlg")
                nc.scalar.activation(tlg[:], te[:], ACT.Ln, bias=1.0)
                tr = sp_pool.tile([P, 512], bf16, tag="spB", name="tr")
                nc.scalar.activation(tr[:], ps[:], ACT.Relu,
                                     bias=dtb_sb[:, blk:blk + 1])
                dchunk = delta[:, blk, tq * 512:(tq + 1) * 512]
                nc.vector.tensor_tensor(dchunk, tr[:], tlg[:], ADD)
            return emit

        def du_group(half, blk):
            def emit():
                h0 = half * H
                nc.vector.tensor_tensor(
                    du[:, blk, h0:h0 + H], delta[:, blk, h0:h0 + H],
                    u16[:, blk, h0:h0 + H], MULT)
            return emit

        def emit_E(half, slots, pace=2):
            """Scan phase for one half. `slots` holds deferred closures of
            the other half's prep work (or the finished half's out_proj),
            drained `pace` per n-iteration so they interleave into each
            engine's FIFO stream."""
            h0 = half * H

            def drain(k):
                for _ in range(k):
                    if slots:
                        slots.pop(0)()

            # carry DMAs emit two iterations late so the SP queue serves
            # the bc prefetch first; gates of the previous blk pair emit a
            # couple of iterations into the next pair so the DVE never
            # stalls waiting for the last y-accumulate at the boundary.
            pend_carry = []
            pend_gate = []

            for bp in range(2):           # blk pairs: (0,1), (2,3)
                blks = (2 * bp, 2 * bp + 1)
                y_ps = {}
                for blk in blks:
                    y_ps[blk] = ps_y.tile([P, H], f32, tag="yps",
                                          name=f"yps{blk}")
                    for c in range(2):
                        nc.tensor.matmul(
                            y_ps[blk][:, c * 512:(c + 1) * 512],
                            diagD[:, blk, :],
                            u16[:, blk, h0 + c * 512:h0 + (c + 1) * 512],
                            start=True, stop=False)
                for n in range(D_STATE):
                    if n == 2 and pend_gate:
                        for g in pend_gate:
                            g()
                        pend_gate = []
                    bc = bc_pool.tile([P, 2, H], bf16, tag="bc", name="bc")
                    nc.sync.dma_start(
                        bc[:],
                        xdbl_s[half][DT_RANK + 2 * n: DT_RANK + 2 * n + 2, :]
                        .partition_broadcast(P))
                    for blk in blks:
                        idx = blk * D_STATE + n
                        dchunk = delta[:, blk, h0:h0 + H]
                        # dA stays fp32: its error compounds over the decay
                        # length, unlike the other bf16-rounded terms.
                        dA = da_pool.tile([P, H], f32, tag="dA", name="dA")
                        nc.scalar.activation(
                            dA[:], dchunk, ACT.Exp,
                            scale=negA[:, idx:idx + 1])
                        dBu = db_pool.tile([P, H], bf16, tag="dBu",
                                           name="dBu")
                        # GpSimd carries most dBu multiplies; DVE takes two
                        # of the sixteen state dims to balance engine load.
                        dbu_eng = nc.vector if n in (5, 11) else nc.gpsimd
                        dbu_eng.tensor_tensor(
                            dBu[:], du[:, blk, h0:h0 + H], bc[:, 0, :], MULT)
                        s = s_pool.tile([P, H], bf16, tag="s", name="s")
                        nc.vector.tensor_tensor_scan(
                            s[:], dA[:], dBu[:],
                            0.0 if half == 0 else carry[:, idx:idx + 1],
                            MULT, ADD)
                        if half == 0:
                            # SBUF->SBUF DMA on the idle SP queue keeps the
                            # carry copy off the Act stream that feeds dA;
                            # deferred so it trails the bc prefetch.
                            pend_carry.append((idx, s))
                        sC = s_pool.tile([P, H], bf16, tag="sC", name="sC")
                        nc.vector.tensor_tensor(sC[:], s[:], bc[:, 1, :],
                                                MULT)
                        for c in range(2):
                            nc.tensor.matmul(
                                y_ps[blk][:, c * 512:(c + 1) * 512],
                                ident16[:],
                                sC[:, c * 512:(c + 1) * 512],
                                start=False, stop=(n == D_STATE - 1))
                    while len(pend_carry) > 2:
                        cidx, cs = pend_carry.pop(0)
                        nc.sync.dma_start(carry[:, cidx:cidx + 1],
                                          cs[:, H - 1:H])
                    drain(pace)
                for cidx, cs in pend_carry:
                    nc.sync.dma_start(carry[:, cidx:cidx + 1],
                                      cs[:, H - 1:H])
                pend_carry = []

                # gate for this blk pair: yg = y * SiLU(z); deferred into
                # the next pair's stream (flushed at its n==2).
                def make_gate(blk, y_tile):
                    def g():
                        nc.vector.tensor_tensor(
                            yg[:, blk, h0:h0 + H], y_tile[:],
                            zsil[:, blk, h0:h0 + H], MULT)
                    return g
                for blk in blks:
                    pend_gate.append(make_gate(blk, y_ps[blk]))
            for g in pend_gate:
                g()
            drain(len(slots))

        def outproj_group(mb, half):
            def emit():
                for c in range(2):
                    tq = half * 2 + c
                    po = ps_all.tile([P, 512], f32, tag="ps", name="po")
                    for kb in range(NBLK):
                        nc.tensor.matmul(
                            po[:], ow_b[:, kb, mb * P:(mb + 1) * P],
                            yg[:, kb, tq * 512:(tq + 1) * 512],
                            start=(kb == 0), stop=(kb == 3))
                    ot = ot_pool.tile([P, 512], bf16, tag="ot", name="ot")
                    nc.scalar.copy(ot[:], po[:])
                    nc.sync.dma_start(
                        outpart[mb * P:(mb + 1) * P,
                                tq * 512:(tq + 1) * 512], ot[:])
            return emit

        # ---------------- program schedule ----------------
        try:
            # Prologue: transpose the whole sequence (PSUM evacuation on the
            # otherwise-idle DVE), then in_proj + conv for BOTH halves: these
            # carry the SiLU ops, which live in a different activation-table
            # set than Exp/Ln — keeping them out of the scan windows avoids
            # per-op table reloads on the scalar engine.
            hsb_es = ExitStack()
            hsb_pool = hsb_es.enter_context(
                tc.tile_pool(name="hsbpool", bufs=2))

            def emit_transposes(tqs):
                for tq in tqs:
                    for kb2 in range(2):
                        psts = [ps_all.tile([P, 512], f32, tag="ps",
                                            name=f"pst{kb}")
                                for kb in range(4)]
                        for tbp in range(2):
                            hsb_sb = hsb_pool.tile(
                                [P, 2, 512], f32, tag="hsb",
                                name=f"hsb{tq}_{kb2}_{tbp}")
                            # alternate trigger queues so descriptor
                            # generation does not serialize on one engine
                            eng = nc.gpsimd if (tq + kb2) % 2 else nc.sync
                            eng.dma_start(
                                hsb_sb[:],
                                hsb[tq * 512 + tbp * 256:
                                    tq * 512 + tbp * 256 + 256,
                                    kb2 * 512:(kb2 + 1) * 512]
                                .rearrange("(tb p) m -> p tb m", p=P))
                            for kb in range(4):
                                for tb in range(2):
                                    c0 = (tbp * 2 + tb) * P
                                    nc.tensor.transpose(
                                        psts[kb][:, c0:c0 + P],
                                        hsb_sb[:, tb, kb * P:(kb + 1) * P],
                                        ident[:])
                        for kb in range(4):
                            nc.vector.tensor_copy(
                                hsT16[:, 4 * kb2 + kb,
                                      tq * 512:(tq + 1) * 512],
                                psts[kb][:])

            emit_transposes((0, 1))

            # Pin the exp+ln table set after the most recent SiLU so the
            # softplus/dA mix never reloads tables (the auto placement pass
            # would otherwise thrash exp<->ln sets). Without the explicit
            # dependency the scheduler hoists the load to program start.
            set_idx = list(get_activation_tables(nc.m.arch)).index(
                "natural_log_exp_and_others")

            def preload_lnexp():
                inst = mybir.InstLoadActFuncSet(
                    name=nc.get_next_instruction_name(),
                    act_func_set_id=set_idx, ins=[], outs=[])
                nc.scalar.add_instruction(inst)
                inst.add_dependency(
                    last_silu[0].ins.name,
                    mybir.DependencyInfo(sync=True, no_sync=False))

            # h0 chain straight through: in_proj -> conv (on the otherwise
            # idle DVE) -> x_dbl -> AllReduce -> dt for the first blk pair,
            # so E(h0) can start while h1's conv still streams on PE
            # behind it. h1's x-side in_proj + conv run in the prologue
            # (their SiLUs must precede the exp/ln table pin); h1's z-side
            # in_proj defers into E(h0) with a table-neutral raw copy.
            # interleave: conv(blk) on the DVE as soon as its x columns are
            # evacuated, while in_proj of later mb's continues on the PE.
            with tc.tile_pool(name="cvpool", bufs=2) as cv_pool:
                cv_box[0] = cv_pool
                for mb in range(8):
                    for tq in (0, 1):
                        inproj_group(tq, mb)()
                    if mb < 4:
                        conv_dve(0, mb)()
                emit_transposes((2, 3))
            hsb_es.close()
            for blk in range(NBLK):
                halo_copy(blk)()
            for tq in (2, 3):
                for mb in range(4):
                    inproj_group(tq, mb)()
            for tq in (0, 1):
                xdbl_group(tq)()
            for blk in range(NBLK):
                for tq in (2, 3):
                    conv_group(tq, blk)()
            emit_ar(0)
            preload_lnexp()
            bc_pool = top.enter_context(tc.tile_pool(name="bcpool", bufs=3))
            da_pool = top.enter_context(tc.tile_pool(name="dapool", bufs=3))
            db_pool = top.enter_context(tc.tile_pool(name="dbpool", bufs=3))
            s_pool = top.enter_context(tc.tile_pool(name="spool", bufs=3))
            for blk in (0, 1):
                for tq in (0, 1):
                    dt_group(tq, blk)()
                du_group(0, blk)()
            if lvl < 3:
                raise _Stop()

            # Remaining prep (all exp/ln/copy-table-friendly), deferred
            # into E(h0)'s engine streams.
            slots = []
            for blk in (2, 3):
                for tq in (0, 1):
                    slots.append(dt_group(tq, blk))
                slots.append(du_group(0, blk))
            for tq in (2, 3):
                for mb in range(4, 8):
                    slots.append(inproj_group(tq, mb, raw_z=True))
            for tq in (2, 3):
                slots.append(xdbl_group(tq))
            slots.append(lambda: emit_ar(1))
            for blk in range(NBLK):
                for tq in (2, 3):
                    slots.append(dt_group(tq, blk))
            for blk in range(NBLK):
                slots.append(du_group(1, blk))

            emit_E(0, slots, pace=1)
            if lvl < 4:
                raise _Stop()

            # E(h1): batch-SiLU the raw z(h1) first (two table loads
            # instead of one per op), re-pin exp/ln, then the scan with
            # out_proj(h0) slotted into the PE/Act gaps.
            for blk in range(NBLK):
                last_silu[0] = nc.scalar.activation(
                    zsil[:, blk, H:], zsil[:, blk, H:], ACT.Silu)
            preload_lnexp()
            emit_E(1, [outproj_group(mb, 0) for mb in range(8)], pace=1)
            if lvl < 5:
                raise _Stop()

            for mb in range(8):
                outproj_group(mb, 1)()

            if collectives:
                nc.gpsimd.collective_compute(
                    "ReduceScatter", ADD, replica_groups=GROUPS,
                    ins=[outpart[:]], outs=[outrs[:]])
            else:
                nc.sync.dma_start(outrs[:], outpart[0:D_MODEL // 4, :])
            nc.sync.dma_start(out_shard[:], outrs[:])

        except _Stop:
            pass

    nc.compile()
    return nc

def _shard_inputs(inputs):
    hs = np.ascontiguousarray(np.asarray(inputs["hidden_states"], np.float32))
    win = np.asarray(inputs["in_proj_w"], np.float32)
    convw = np.asarray(inputs["conv_w"], np.float32)
    convb = np.asarray(inputs["conv_b"], np.float32)
    xproj = np.asarray(inputs["x_proj_w"], np.float32)
    dtw = np.asarray(inputs["dt_proj_w"], np.float32)
    dtb = np.asarray(inputs["dt_proj_b"], np.float32)
    alog = np.asarray(inputs["A_log"], np.float32)
    dv = np.asarray(inputs["D"], np.float32)
    outw = np.asarray(inputs["out_proj_w"], np.float32)

    # permute x_proj rows so B_n/C_n come out interleaved: one broadcast DMA
    # per state-dim fetches both rows.
    perm = list(range(DT_RANK))
    for n in range(D_STATE):
        perm.append(DT_RANK + n)            # B_n -> row 64+2n
        perm.append(DT_RANK + D_STATE + n)  # C_n -> row 65+2n
    xproj_p = xproj[perm]

    in_maps = []
    for c in range(NCORES):
        b, k = c // 4, c % 4
        cs, ce = k * DC, (k + 1) * DC
        wxT = win[cs:ce].T            # [1024, 512]
        wzT = win[D_INNER + cs:D_INNER + ce].T
        wcat = np.concatenate([wxT, wzT], axis=1)  # [1024, 1024]
        wxz = np.ascontiguousarray(
            wcat.reshape(8, P, 1024).transpose(1, 0, 2))  # [128, 8, 1024]
        in_maps.append({
            "hsb": hs[b],
            "wxz": wxz,
            "convw": np.ascontiguousarray(
                convw[cs:ce].reshape(NBLK, P, D_CONV).transpose(1, 0, 2)
                .reshape(P, NBLK * D_CONV)),
            "convb": np.ascontiguousarray(convb[cs:ce].reshape(NBLK, P).T),
            "xprojT": np.ascontiguousarray(
                xproj_p[:, cs:ce].T.reshape(NBLK, P, 96).transpose(1, 0, 2)),
            "dtprojT": np.ascontiguousarray(dtw[cs:ce].T),
            "dtb": np.ascontiguousarray(dtb[cs:ce].reshape(NBLK, P).T),
            "alog": np.ascontiguousarray(
                alog[cs:ce].reshape(NBLK, P, D_STATE).transpose(1, 0, 2)
                .reshape(P, NBLK * D_STATE)),
            "dvec": np.ascontiguousarray(dv[cs:ce].reshape(NBLK, P).T),
            "outw": np.ascontiguousarray(
                outw[:, cs:ce].T.reshape(NBLK, P, 1024)
                .transpose(1, 0, 2)),
        })
    return in_maps


def kernel(**inputs):
    global LAST_RESULTS
    from concourse.bass_utils import run_bass_kernel_spmd

    if "prog" not in _prog_cache:
        _prog_cache["prog"] = _build_program()
    nc = _prog_cache["prog"]

    in_maps = _shard_inputs(inputs)
    res = run_bass_kernel_spmd(nc, in_maps, list(range(NCORES)),
                               trace=TRACE)
    LAST_RESULTS = res

    out = np.empty((BATCH, L, D_MODEL), np.float32)
    for g in range(BATCH):
        rows = np.concatenate(
            [res.results[g * 4 + i]["out_shard"] for i in range(4)], axis=0)
        out[g] = rows.T
    return out

